# revision 1
# baseline (speedup 1.0000x reference)
"""GQA decode attention with paged KV cache on 8 TRN2 NeuronCores.

Sharding: tensor-parallel over the 8 KV heads (one head per core).
Each core gets host-pre-transposed shards:
  kt  (8, 128, 4096)  K^T pair-packed: kt[p, par*64+d, l] = K[b=2p+par, l, d]
  v   (16, 128, 2048) V chunk-major:   v[b, pl, n*64+d]   = V[b, l=n*128+pl, d]
  wqt (16, 128, 256)  Wq[h*256+j, :].T k-chunked
  wkt (16, 128, 64)   Wk[h*64+d, :].T k-chunked
  wvt (16, 128, 64)
  wot (2, 128, 2048)  Wo[:, h*256+j].T j-chunked
  xt  (16, 128, 16)   x.T k-chunked
Output per core: outt (2048, 16) partial out.T; host sums 8 partials.

Score rows live in a sparse 32-aligned layout (engine APs require base
partition in {0,32,64,96}): batch-pair p -> psum-half h=p//4, 32-row
block r32=p%4, rows 32*r32..32*r32+7 valid. The strided probsT copies
collapse this back to the dense (l, b*4+gi) layout used by PV.
"""

import numpy as np
from contextlib import ExitStack

import concourse.bass as bass
import concourse.mybir as mybir
import concourse.tile as tile
from concourse import bacc
from concourse.masks import make_identity

F32 = mybir.dt.float32
F32R = mybir.dt.float32r
EXP = mybir.ActivationFunctionType.Exp

B = 16          # batch (decode requests)
NPAIR = 8       # batch pairs
L = 4096        # padded cache length (NB*TB)
HD = 64         # head dim
G = 4           # GQA group size
EMB = 2048
KC = 16         # 128-contraction chunks over EMB
NC512 = 8       # 512-wide l-chunks
NC128 = 32      # 128-wide l-chunks
BG = 64         # B*G score rows
N_CORES = 8


def build_bass(lvalid: int):
    nc = bacc.Bacc(
        "TRN2",
        target_bir_lowering=False,
        debug=False,
        enable_asserts=False,
        num_devices=N_CORES,
    )
    kt = nc.dram_tensor("kt", (NPAIR, 128, L), F32, kind="ExternalInput").ap()
    vv = nc.dram_tensor("v", (B, 128, 2048), F32, kind="ExternalInput").ap()
    wqt = nc.dram_tensor("wqt", (KC, 128, 256), F32, kind="ExternalInput").ap()
    wkt = nc.dram_tensor("wkt", (KC, 128, HD), F32, kind="ExternalInput").ap()
    wvt = nc.dram_tensor("wvt", (KC, 128, HD), F32, kind="ExternalInput").ap()
    wot = nc.dram_tensor("wot", (2, 128, EMB), F32, kind="ExternalInput").ap()
    xt = nc.dram_tensor("xt", (KC, 128, B), F32, kind="ExternalInput").ap()
    outt = nc.dram_tensor("outt", (EMB, B), F32, kind="ExternalOutput").ap()

    with tile.TileContext(nc) as tc, ExitStack() as ctx:
        sb_const = ctx.enter_context(tc.tile_pool(name="const", bufs=1))
        sb_w = ctx.enter_context(tc.tile_pool(name="w", bufs=16))
        sb_k = ctx.enter_context(tc.tile_pool(name="k", bufs=16))
        sb_v = ctx.enter_context(tc.tile_pool(name="v", bufs=4))
        sb_p = ctx.enter_context(tc.tile_pool(name="p", bufs=1))
        sb_o = ctx.enter_context(tc.tile_pool(name="o", bufs=2))
        ps_a = ctx.enter_context(tc.tile_pool(name="psa", bufs=4, space="PSUM"))
        ps_b = ctx.enter_context(tc.tile_pool(name="psb", bufs=4, space="PSUM"))

        ident = sb_const.tile([128, 128], F32, tag="ident")
        make_identity(nc, ident[:])

        # ---- persistent sbuf tiles ----
        qbig = sb_p.tile([128, 64], F32, tag="qbig")    # block-diag q/8
        kcur = sb_p.tile([128, NPAIR], F32, tag="kcur")
        vcurT = sb_p.tile([HD, B], F32, tag="vcurT")
        vcur_wide = sb_p.tile([1, B * HD], F32, tag="vcurw")
        scur_exp = sb_p.tile([128, 2], F32, tag="scur")     # sparse rows
        scur_row0 = sb_p.tile([1, 128], F32, tag="scurr0")
        scur_row1 = sb_p.tile([1, 128], F32, tag="scurr1")
        sexp = sb_p.tile([128, 2 * L], F32, tag="sexp")     # sparse rows
        probsT = sb_p.tile([128, 2048], F32, tag="probsT")  # dense (l, bg)
        sums = sb_p.tile([128, 2], F32, tag="sums")
        sums2 = sb_p.tile([128, 2], F32, tag="sums2")
        recip = sb_p.tile([128, 2], F32, tag="recip")
        oT = sb_p.tile([HD, BG], F32, tag="oT")
        wo_g0 = sb_p.tile([HD, EMB], F32, tag="wo0")
        wo_g1 = sb_p.tile([HD, EMB], F32, tag="wo1")
        wo_g2 = sb_p.tile([HD, EMB], F32, tag="wo2")
        wo_g3 = sb_p.tile([HD, EMB], F32, tag="wo3")
        wo_g = [wo_g0, wo_g1, wo_g2, wo_g3]

        nc.vector.memset(qbig[:], 0.0)

        # ---- phase 1: projections q, k_cur, v_cur ----
        qbig_ps = ps_b.tile([128, 32], F32, tag="b")
        kcur_ps = ps_b.tile([128, NPAIR], F32, tag="b")
        vcur_ps = ps_b.tile([HD, B], F32, tag="b")
        wq_ts, wk_ts, wv_ts, x_ts = [], [], [], []
        for kc in range(KC):
            wq_t = sb_w.tile([128, 256], F32, tag="wq")
            nc.sync.dma_start(wq_t[:], wqt[kc])
            wk_t = sb_w.tile([128, HD], F32, tag="wk")
            nc.sync.dma_start(wk_t[:], wkt[kc])
            wv_t = sb_w.tile([128, HD], F32, tag="wv")
            nc.sync.dma_start(wv_t[:], wvt[kc])
            x_t = sb_w.tile([128, B], F32, tag="x")
            nc.sync.dma_start(x_t[:], xt[kc])
            wq_ts.append(wq_t)
            wk_ts.append(wk_t)
            wv_ts.append(wv_t)
            x_ts.append(x_t)
        # sequential accumulation groups: each psum region's full K loop
        # runs contiguously (the sim forbids interleaved groups per bank)
        for gi in range(G):
            for par in range(2):
                for kc in range(KC):
                    nc.tensor.matmul(
                        qbig_ps[par * 64:par * 64 + 64, gi * 8:gi * 8 + 8],
                        wq_ts[kc][:, gi * 64:gi * 64 + 64],
                        x_ts[kc][:, par::2],
                        start=(kc == 0), stop=(kc == KC - 1),
                        skip_group_check=True)
        for par in range(2):
            for kc in range(KC):
                nc.tensor.matmul(
                    kcur_ps[par * 64:par * 64 + 64, :],
                    wk_ts[kc][:], x_ts[kc][:, par::2],
                    start=(kc == 0), stop=(kc == KC - 1),
                    skip_group_check=True)
        for kc in range(KC):
            nc.tensor.matmul(
                vcur_ps[:], wv_ts[kc][:], x_ts[kc][:],
                start=(kc == 0), stop=(kc == KC - 1),
                skip_group_check=True)

        # qbig[par*64+d, p*8 + par*4 + gi] = q[2p+par, gi, d] / 8
        for gi in range(G):
            for par in range(2):
                nc.vector.tensor_scalar_mul(
                    qbig[par * 64:par * 64 + 64, (par * 4 + gi)::8],
                    qbig_ps[par * 64:par * 64 + 64, gi * 8:gi * 8 + 8],
                    0.125)
        nc.vector.tensor_copy(kcur[:], kcur_ps[:])
        nc.vector.tensor_copy(vcurT[:], vcur_ps[:])
        # vcur_wide[0, b*64+d] = v_cur[b, d]
        for b in range(B):
            vw_ps = ps_b.tile([1, HD], F32, tag="b")
            nc.tensor.transpose(vw_ps[:], vcurT[:, b:b + 1], ident[0:HD, 0:HD])
            nc.vector.tensor_copy(vcur_wide[:, b * HD:(b + 1) * HD], vw_ps[:])

        # ---- current-token scores (sparse 32-aligned rows) ----
        scur_ps = ps_b.tile([128, 2], F32, tag="b")
        nc.vector.memset(scur_ps[:], 0.0)
        for p in range(NPAIR):
            r0, h = 32 * (p % 4), p // 4
            nc.tensor.matmul(
                scur_ps[r0:r0 + 8, h:h + 1],
                qbig[:, p * 8:p * 8 + 8], kcur[:, p:p + 1],
                start=True, stop=True, tile_position=(0, r0),
                skip_group_check=True)
        nc.scalar.activation(scur_exp[:], scur_ps[:], EXP)

        # ---- phase 2: cached scores + exp ----
        for c in range(NC512):
            lo, hi = c * 512, min((c + 1) * 512, lvalid)
            for h in range(2):
                s_ps = ps_a.tile([128, 512], F32, tag="a")
                nc.vector.memset(s_ps[:], 0.0)
                for r32 in range(4):
                    p = h * 4 + r32
                    k_ct = sb_k.tile([128, 512], F32, tag="k")
                    nc.sync.dma_start(k_ct[:], kt[p, :, c * 512:(c + 1) * 512])
                    nc.tensor.matmul(
                        s_ps[32 * r32:32 * r32 + 8, :],
                        qbig[:, p * 8:p * 8 + 8],
                        k_ct[:],
                        start=True, stop=True, tile_position=(0, 32 * r32),
                        skip_group_check=True)
                if hi > lo:
                    nc.scalar.activation(
                        sexp[:, h * L + lo:h * L + hi], s_ps[:, 0:hi - lo], EXP)
        if lvalid < L:
            for h in range(2):
                nc.vector.memset(sexp[:, h * L + lvalid:(h + 1) * L], 0.0)

        # ---- phase 3: softmax normalization ----
        for h in range(2):
            nc.vector.reduce_sum(
                sums[:, h:h + 1], sexp[:, h * L:(h + 1) * L],
                axis=mybir.AxisListType.X)
        nc.vector.tensor_add(sums2[:], sums[:], scur_exp[:])
        nc.vector.reciprocal(recip[:], sums2[:])
        for h in range(2):
            nc.vector.tensor_scalar_mul(
                sexp[:, h * L:(h + 1) * L], sexp[:, h * L:(h + 1) * L],
                recip[:, h:h + 1])
        nc.vector.tensor_mul(scur_exp[:], scur_exp[:], recip[:])
        for h, srow in ((0, scur_row0), (1, scur_row1)):
            sr_ps = ps_b.tile([1, 128], F32, tag="b")
            nc.tensor.transpose(sr_ps[:], scur_exp[:, h:h + 1], ident[:])
            nc.vector.tensor_copy(srow[:], sr_ps[:])

        # ---- phase 4: transpose probs to dense l-major layout ----
        # src sparse col r = 32*r32 + m (m = 4*par+gi); dst col = c*64 + bg
        # where bg = 32*h + 8*r32 + m.
        for c in range(NC128):
            for h in range(2):
                t_ps = ps_a.tile([128, 128], F32, tag="a")
                nc.tensor.transpose(
                    t_ps[:], sexp[:, h * L + c * 128:h * L + (c + 1) * 128],
                    ident[:])
                src = t_ps[:].rearrange("p (a m) -> p a m", a=4)[:, :, 0:8]
                dst = probsT[:, c * 64 + 32 * h:c * 64 + 32 * h + 32]
                dst = dst.rearrange("p (a m) -> p a m", a=4)
                nc.vector.tensor_copy(dst, src)

        # ---- phase 5: PV ----
        oT_ps = ps_b.tile([HD, BG], F32, tag="b")
        for b in range(B):
            v_t = sb_v.tile([128, 2048], F32, tag="v")
            for n4 in range(4):
                nc.sync.dma_start(
                    v_t[:, n4 * 512:(n4 + 1) * 512],
                    vv[b, :, n4 * 512:(n4 + 1) * 512])
            for c in range(NC128):
                nc.tensor.matmul(
                    oT_ps[:, b * 4:b * 4 + 4],
                    v_t[:, c * 64:(c + 1) * 64],
                    probsT[:, c * 64 + b * 4:c * 64 + b * 4 + 4],
                    start=(c == 0), stop=False, skip_group_check=True)
            h, rem = b // 8, b % 8
            srow = scur_row0 if h == 0 else scur_row1
            nc.tensor.matmul(
                oT_ps[:, b * 4:b * 4 + 4],
                vcur_wide[:, b * HD:(b + 1) * HD],
                srow[:, 32 * (rem // 2) + 4 * (rem % 2):
                     32 * (rem // 2) + 4 * (rem % 2) + 4],
                start=False, stop=True, skip_group_check=True)
        nc.vector.tensor_copy(oT[:], oT_ps[:])

        # ---- phase 6: Wo projection ----
        for gi in range(G):
            for q4 in range(4):
                nc.sync.dma_start(
                    wo_g[gi][:, q4 * 512:(q4 + 1) * 512],
                    wot[gi // 2, (gi % 2) * 64:(gi % 2) * 64 + 64,
                        q4 * 512:(q4 + 1) * 512])
        for cc in range(KC):
            ot_ps = ps_b.tile([128, B], F32, tag="b")
            for gi in range(G):
                nc.tensor.matmul(
                    ot_ps[:],
                    wo_g[gi][:, cc * 128:(cc + 1) * 128],
                    oT[:, gi::4],
                    start=(gi == 0), stop=(gi == 3))
            o_sb = sb_o.tile([128, B], F32, tag="o")
            nc.vector.tensor_copy(o_sb[:], ot_ps[:])
            nc.sync.dma_start(outt[cc * 128:(cc + 1) * 128, :], o_sb[:])

    nc.compile()
    return nc


def make_in_maps(x, blocks_k, blocks_v, Wq, Wk, Wv, Wo):
    x2 = np.asarray(x, np.float32).reshape(B, EMB)
    xt_h = np.ascontiguousarray(x2.T).reshape(KC, 128, B)
    in_maps = []
    for h in range(N_CORES):
        bk = np.asarray(blocks_k[:, :, h], np.float32)   # (NB, B, TB, HD)
        kt_h = np.ascontiguousarray(
            bk.transpose(1, 3, 0, 2).reshape(B, HD, L)
        ).reshape(NPAIR, 128, L)
        bv = np.asarray(blocks_v[:, :, h], np.float32)
        vlin = bv.transpose(1, 0, 2, 3).reshape(B, L, HD)
        v_h = np.ascontiguousarray(
            vlin.reshape(B, NC128, 128, HD).transpose(0, 2, 1, 3)
        ).reshape(B, 128, 2048)
        wq_h = np.ascontiguousarray(
            np.asarray(Wq, np.float32)[h * 256:(h + 1) * 256].T
        ).reshape(KC, 128, 256)
        wk_h = np.ascontiguousarray(
            np.asarray(Wk, np.float32)[h * 64:(h + 1) * 64].T
        ).reshape(KC, 128, HD)
        wv_h = np.ascontiguousarray(
            np.asarray(Wv, np.float32)[h * 64:(h + 1) * 64].T
        ).reshape(KC, 128, HD)
        wo_h = np.ascontiguousarray(
            np.asarray(Wo, np.float32)[:, h * 256:(h + 1) * 256].T
        ).reshape(2, 128, EMB)
        in_maps.append(dict(
            kt=np.ascontiguousarray(kt_h),
            v=np.ascontiguousarray(v_h),
            wqt=wq_h, wkt=wk_h, wvt=wv_h, wot=wo_h,
            xt=np.ascontiguousarray(xt_h)))
    return in_maps


_cache = {}


def get_bass(lvalid: int):
    if lvalid not in _cache:
        _cache[lvalid] = build_bass(lvalid)
    return _cache[lvalid]


def kernel(x, blocks_k, blocks_v, Wq, Wk, Wv, Wo, last_offset):
    from concourse import bass_utils

    lvalid = 15 * 256 + int(last_offset)
    nc = get_bass(lvalid)
    in_maps = make_in_maps(x, blocks_k, blocks_v, Wq, Wk, Wv, Wo)
    res = bass_utils.run_bass_kernel_spmd(
        nc, in_maps, core_ids=list(range(N_CORES)))
    total = np.zeros((EMB, B), np.float64)
    for r in res.results:
        total += r["outt"].astype(np.float64)
    return np.ascontiguousarray(total.T.astype(np.float32)).reshape(B, 1, EMB)

